# revision 5
# baseline (speedup 1.0000x reference)
"""Trainium2 Bass kernel for nn_DecoderPromptLayerWithNMR.

Sharding: 8 fully-independent shards, core = (batch b in 0..3, query-half j in 0..1).
Each core computes the full layer for 512 target queries of one batch element.
No collectives. Feature-major attention with softmax-denominator via ones-column.
"""
import sys
import os

sys.path.insert(0, "/opt/trn_rl_repo")

import numpy as np
import ml_dtypes

import concourse.bass as bass
from concourse import bacc, mybir
from concourse.tile import TileContext

F32 = mybir.dt.float32
BF16 = mybir.dt.bfloat16
AF = mybir.ActivationFunctionType
OP = mybir.AluOpType

# Problem dims
D = 1024
H = 16
DH = 64
ROT = 32
FFN = 4096
B, T, M, N = 4, 1024, 128, 64
PREF = M + N            # 192
PAD_PREF = 256          # padded prefix (64 zero rows, masked)
LP = PAD_PREF + T       # 1280 padded key length
KK = LP // 128          # 10 key tiles
TQ = T // 2             # 512 queries per core
QT = TQ // 128          # 4 query token tiles
DT = D // 128           # 8 feature tiles
FT = FFN // 128         # 32 ffn tiles
EPS = 1e-5


def build_kernel():
    nc = bacc.Bacc(None, target_bir_lowering=False)

    xin = nc.declare_dram_parameter("xin", [LP, D], F32, isOutput=False)
    xq = nc.declare_dram_parameter("xq", [TQ, D], F32, isOutput=False)
    wq = nc.declare_dram_parameter("wq", [D, D], BF16, isOutput=False)
    wk = nc.declare_dram_parameter("wk", [D, D], BF16, isOutput=False)
    wv = nc.declare_dram_parameter("wv", [D, D], BF16, isOutput=False)
    wo = nc.declare_dram_parameter("wo", [D, D], BF16, isOutput=False)
    w1 = nc.declare_dram_parameter("w1", [D, FFN], BF16, isOutput=False)
    w2 = nc.declare_dram_parameter("w2", [FFN, D], BF16, isOutput=False)
    bq = nc.declare_dram_parameter("bq", [128, DT], F32, isOutput=False)
    bk = nc.declare_dram_parameter("bk", [128, DT], F32, isOutput=False)
    bv_r = nc.declare_dram_parameter("bv_r", [1, D], BF16, isOutput=False)
    bo = nc.declare_dram_parameter("bo", [128, DT], F32, isOutput=False)
    b1 = nc.declare_dram_parameter("b1", [128, FT], F32, isOutput=False)
    b2 = nc.declare_dram_parameter("b2", [128, DT], F32, isOutput=False)
    cosq = nc.declare_dram_parameter("cosq", [128, TQ], BF16, isOutput=False)
    sinq = nc.declare_dram_parameter("sinq", [128, TQ], BF16, isOutput=False)
    cosk = nc.declare_dram_parameter("cosk", [128, LP], BF16, isOutput=False)
    sink = nc.declare_dram_parameter("sink", [128, LP], BF16, isOutput=False)
    maskt = nc.declare_dram_parameter("maskt", [128, KK, TQ], BF16, isOutput=False)
    yout = nc.declare_dram_parameter("y", [TQ, D], F32, isOutput=True)

    # DRAM scratch for transpose bounces
    s_xa = nc.dram_tensor("s_xa", [LP, D], BF16)
    s_xqa = nc.dram_tensor("s_xqa", [TQ, D], BF16)
    s_yT = nc.dram_tensor("s_yT", [D, TQ], BF16)
    s_x2 = nc.dram_tensor("s_x2", [TQ, D], BF16)
    s_y2T = nc.dram_tensor("s_y2T", [D, TQ], BF16)

    with TileContext(nc) as tc:
        with tc.tile_pool(name="persist", bufs=1) as persist, \
             tc.tile_pool(name="stats", bufs=4) as statsp:

            eps_t = persist.tile([128, 1], F32)
            nc.vector.memset(eps_t, EPS)
            ones_t = persist.tile([1, 128], BF16)
            nc.vector.memset(ones_t, 1.0)
            ones_f = persist.tile([1, 64], F32)
            nc.vector.memset(ones_f, 1.0)

            # persistent activation tensors (live until end)
            kT = persist.tile([128, DT, LP], BF16)      # K^T (rope'd)
            qT = persist.tile([128, DT, TQ], BF16)      # Q^T (rope'd)
            v_sb = persist.tile([128, KK, H, DH + 1], BF16)  # V token-major + ones
            attnT = persist.tile([128, DT, TQ], BF16)   # normalized attn^T
            x1 = persist.tile([128, QT, D], F32)        # attn-out + residual
            mask_sb = persist.tile([128, KK, TQ], BF16)

            nc.sync.dma_start(mask_sb[:], maskt[:])
            nc.vector.memset(v_sb[:, :, :, DH:DH + 1], 1.0)

            def ln_stats(src_ap):
                """src_ap: [128, 1024] fp32 -> (mean, rstd) [128,1] each."""
                st = statsp.tile([128, 2, 6], F32, tag="bn")
                nc.vector.bn_stats(out=st[:, 0, :], in_=src_ap[:, 0:512])
                nc.vector.bn_stats(out=st[:, 1, :], in_=src_ap[:, 512:1024])
                mv = statsp.tile([128, 2], F32, tag="mv")
                nc.vector.bn_aggr(out=mv[:], in_=st[:])
                rstd = statsp.tile([128, 1], F32, tag="rstd")
                nc.scalar.activation(out=rstd[:], in_=mv[:, 1:2],
                                     func=AF.Sqrt, bias=eps_t[:], scale=1.0)
                nc.vector.reciprocal(out=rstd[:], in_=rstd[:])
                return mv, rstd

            with tc.tile_pool(name="pa", bufs=1) as pa:
                xaT = pa.tile([128, DT, LP], BF16)     # LN1(xin)^T feature-major
                xqT = pa.tile([128, DT, TQ], BF16)     # LN1(xq)^T
                cosq_sb = pa.tile([128, TQ], BF16)
                sinq_sb = pa.tile([128, TQ], BF16)
                cosk_sb = pa.tile([128, LP], BF16)
                sink_sb = pa.tile([128, LP], BF16)
                nc.sync.dma_start(cosq_sb[:], cosq[:])
                nc.sync.dma_start(sinq_sb[:], sinq[:])
                nc.sync.dma_start(cosk_sb[:], cosk[:])
                nc.sync.dma_start(sink_sb[:], sink[:])

                # ---------- Phase 1: LN1 -> bf16 DRAM scratch ----------------
                with tc.tile_pool(name="ln", bufs=3) as lnp:
                    for src, n_tiles, dst in ((xin, KK, s_xa), (xq, QT, s_xqa)):
                        for t in range(n_tiles):
                            x_t = lnp.tile([128, D], F32, tag="ln_in")
                            nc.sync.dma_start(x_t[:], src[t * 128:(t + 1) * 128, :])
                            mv, rstd = ln_stats(x_t[:])
                            xa_t = lnp.tile([128, D], BF16, tag="ln_out")
                            nc.vector.tensor_scalar(
                                out=xa_t[:], in0=x_t[:],
                                scalar1=mv[:, 0:1], scalar2=rstd[:],
                                op0=OP.subtract, op1=OP.mult)
                            nc.sync.dma_start(dst[t * 128:(t + 1) * 128, :], xa_t[:])

                # ---------- Phase 2: transpose to feature-major --------------
                for f in range(DT):
                    nc.sync.dma_start_transpose(
                        xaT[:, f, :], s_xa[:, f * 128:(f + 1) * 128])
                    nc.sync.dma_start_transpose(
                        xqT[:, f, :], s_xqa[:, f * 128:(f + 1) * 128])

                # ---------- Phase 3: K/Q/V projections + RoPE ----------------
                def rope(dst, m, ntok, cos_sb, sin_sb, pool):
                    # sin table carries the rotate-half sign; shifts via DMA
                    rot = pool.tile([128, ntok], BF16, tag="rot")
                    nc.vector.memset(rot[32:64, :], 0.0)
                    nc.vector.memset(rot[96:128, :], 0.0)
                    nc.sync.dma_start(rot[0:16, :], dst[16:32, m, :])
                    nc.sync.dma_start(rot[16:32, :], dst[0:16, m, :])
                    nc.sync.dma_start(rot[64:80, :], dst[80:96, m, :])
                    nc.sync.dma_start(rot[80:96, :], dst[64:80, m, :])
                    nc.vector.tensor_tensor(rot[:], rot[:], sin_sb[:, :ntok], OP.mult)
                    nc.vector.tensor_tensor(dst[:, m, :], dst[:, m, :],
                                            cos_sb[:, :ntok], OP.mult)
                    nc.vector.tensor_tensor(dst[:, m, :], dst[:, m, :], rot[:], OP.add)

                with tc.tile_pool(name="wqkv", bufs=2) as wp, \
                     tc.tile_pool(name="qkv_ps", bufs=2, space="PSUM") as pps, \
                     tc.tile_pool(name="qkv_tmp", bufs=3) as tp:
                    bias_sb = wp.tile([128, 3 * DT], F32, tag="bias")
                    nc.sync.dma_start(bias_sb[:, 0:DT], bk[:])
                    nc.sync.dma_start(bias_sb[:, DT:2 * DT], bq[:])
                    bvr_sb = wp.tile([1, D], BF16, tag="bvr")
                    nc.sync.dma_start(bvr_sb[:], bv_r[:])

                    # K^T [hd, LP]
                    wk_sb = wp.tile([128, DT, D], BF16, tag="w")
                    nc.sync.dma_start(
                        wk_sb[:], wk.rearrange("(ko p) n -> p ko n", p=128))
                    for m in range(DT):
                        for c0 in range(0, LP, 512):
                            cw = min(512, LP - c0)
                            ps = pps.tile([128, 512], F32, tag="kq_ps")
                            for k in range(DT):
                                nc.tensor.matmul(
                                    ps[:, :cw],
                                    lhsT=wk_sb[:, k, m * 128:(m + 1) * 128],
                                    rhs=xaT[:, k, c0:c0 + cw],
                                    start=(k == 0), stop=(k == DT - 1))
                            nc.vector.tensor_scalar_add(
                                kT[:, m, c0:c0 + cw], ps[:, :cw],
                                bias_sb[:, m:m + 1])
                        rope(kT, m, LP, cosk_sb, sink_sb, tp)

                    # Q^T [hd, TQ]
                    wq_sb = wp.tile([128, DT, D], BF16, tag="w")
                    nc.sync.dma_start(
                        wq_sb[:], wq.rearrange("(ko p) n -> p ko n", p=128))
                    for m in range(DT):
                        ps = pps.tile([128, 512], F32, tag="kq_ps")
                        for k in range(DT):
                            nc.tensor.matmul(
                                ps[:], lhsT=wq_sb[:, k, m * 128:(m + 1) * 128],
                                rhs=xqT[:, k, :],
                                start=(k == 0), stop=(k == DT - 1))
                        nc.vector.tensor_scalar_add(
                            qT[:, m, :], ps[:], bias_sb[:, DT + m:DT + m + 1])
                        rope(qT, m, TQ, cosq_sb, sinq_sb, tp)

                    # V token-major [kk, dv], bias via rank-1 matmul
                    wv_sb = wp.tile([128, DT, D], BF16, tag="w")
                    nc.sync.dma_start(
                        wv_sb[:], wv.rearrange("(ko p) n -> p ko n", p=128))
                    for kk in range(KK):
                        ps = pps.tile([128, 1024], F32, tag="v_ps")
                        for c0 in (0, 512):
                            for k in range(DT):
                                nc.tensor.matmul(
                                    ps[:, c0:c0 + 512],
                                    lhsT=xaT[:, k, kk * 128:(kk + 1) * 128],
                                    rhs=wv_sb[:, k, c0:c0 + 512],
                                    start=(k == 0), stop=False)
                            nc.tensor.matmul(
                                ps[:, c0:c0 + 512],
                                lhsT=ones_t[:],
                                rhs=bvr_sb[0:1, c0:c0 + 512],
                                start=False, stop=True)
                        nc.vector.tensor_copy(
                            out=v_sb[:, kk, :, 0:DH],
                            in_=ps[:].rearrange("p (h d) -> p h d", h=H))

            # ---------- Phase 4: attention -----------------------------------
            with tc.tile_pool(name="sc_ps", bufs=2, space="PSUM") as scps, \
                 tc.tile_pool(name="at_ps", bufs=1, space="PSUM") as atps, \
                 tc.tile_pool(name="bc_ps", bufs=2, space="PSUM") as bcps, \
                 tc.tile_pool(name="att_tmp", bufs=3) as atp:
                for hg in range(H // 2):
                    ap0 = atps.tile([DH + 1, TQ], F32, tag="ap0")
                    ap1 = atps.tile([DH + 1, TQ], F32, tag="ap1")
                    aps = (ap0, ap1)
                    for kk in range(KK):
                        sp = scps.tile([128, 1024], F32, tag="scores")
                        for i in range(2):
                            r0 = 64 * i
                            nc.tensor.matmul(
                                sp[:, i * 512:(i + 1) * 512],
                                lhsT=kT[r0:r0 + 64, hg, kk * 128:(kk + 1) * 128],
                                rhs=qT[r0:r0 + 64, hg, :],
                                start=True, stop=True)
                        pexp = atp.tile([128, 2, 512], BF16, tag="pexp")
                        nc.scalar.activation(
                            out=pexp[:].rearrange("p a b -> p (a b)"),
                            in_=sp[:], func=AF.Exp, scale=0.125)
                        nc.vector.tensor_tensor(
                            pexp[:],
                            pexp[:],
                            mask_sb[:, kk, None, :].to_broadcast([128, 2, TQ]),
                            OP.mult)
                        for i in range(2):
                            nc.tensor.matmul(
                                aps[i][:],
                                lhsT=v_sb[:, kk, 2 * hg + i, :],
                                rhs=pexp[:, i, :],
                                start=(kk == 0), stop=(kk == KK - 1))
                    for i in range(2):
                        r_sb = atp.tile([1, TQ], F32, tag="recip")
                        nc.vector.reciprocal(r_sb[:], aps[i][DH:DH + 1, :])
                        bps = bcps.tile([64, TQ], F32, tag="bc")
                        nc.tensor.matmul(bps[:], lhsT=ones_f[:],
                                         rhs=r_sb[:], start=True, stop=True)
                        bsb = atp.tile([64, TQ], F32, tag="bcs")
                        nc.scalar.copy(bsb[:], bps[:])
                        nc.vector.tensor_tensor(
                            attnT[64 * i:64 * i + 64, hg, :],
                            aps[i][0:DH, :], bsb[:], OP.mult)

            # ---------- Phase 5: out-proj + residual -------------------------
            with tc.tile_pool(name="wo_p", bufs=1) as wop, \
                 tc.tile_pool(name="o_ps", bufs=2, space="PSUM") as ops, \
                 tc.tile_pool(name="o_tmp", bufs=3) as otp:
                wo_sb = wop.tile([128, DT, D], BF16)
                bo_sb = wop.tile([128, DT], F32)
                nc.sync.dma_start(wo_sb[:], wo.rearrange("(ko p) n -> p ko n", p=128))
                nc.sync.dma_start(bo_sb[:], bo[:])
                for m in range(DT):
                    ps = ops.tile([128, 512], F32, tag="o_ps")
                    for k in range(DT):
                        nc.tensor.matmul(
                            ps[:], lhsT=wo_sb[:, k, m * 128:(m + 1) * 128],
                            rhs=attnT[:, k, :],
                            start=(k == 0), stop=(k == DT - 1))
                    yt = otp.tile([128, 512], BF16, tag="yt")
                    nc.vector.tensor_scalar_add(yt[:], ps[:], bo_sb[:, m:m + 1])
                    nc.sync.dma_start(s_yT[m * 128:(m + 1) * 128, :], yt[:])
                for t in range(QT):
                    y_b = otp.tile([128, D], BF16, tag="y_b")
                    nc.sync.dma_start_transpose(
                        y_b[:], s_yT[:, t * 128:(t + 1) * 128])
                    xq_t = otp.tile([128, D], F32, tag="xq_t")
                    nc.sync.dma_start(xq_t[:], xq[t * 128:(t + 1) * 128, :])
                    nc.vector.tensor_tensor(x1[:, t, :], y_b[:], xq_t[:], OP.add)

            # ---------- Phase 6: LN2 + FFN -----------------------------------
            with tc.tile_pool(name="ffn_tmp", bufs=3) as fp, \
                 tc.tile_pool(name="ffn_w", bufs=3) as fwp, \
                 tc.tile_pool(name="ffn_ps", bufs=2, space="PSUM") as fps, \
                 tc.tile_pool(name="h_pool", bufs=1) as hp:
                for t in range(QT):
                    mv, rstd = ln_stats(x1[:, t, :])
                    x2_t = fp.tile([128, D], BF16, tag="x2_t")
                    nc.vector.tensor_scalar(
                        out=x2_t[:], in0=x1[:, t, :],
                        scalar1=mv[:, 0:1], scalar2=rstd[:],
                        op0=OP.subtract, op1=OP.mult)
                    nc.sync.dma_start(s_x2[t * 128:(t + 1) * 128, :], x2_t[:])
                x2T = hp.tile([128, DT, TQ], BF16)
                for f in range(DT):
                    nc.sync.dma_start_transpose(
                        x2T[:, f, :], s_x2[:, f * 128:(f + 1) * 128])

                b1_sb = hp.tile([128, FT], F32)
                b2_sb = hp.tile([128, DT], F32)
                nc.sync.dma_start(b1_sb[:], b1[:])
                nc.sync.dma_start(b2_sb[:], b2[:])

                h_sb = hp.tile([128, FT, TQ], BF16)
                for m in range(FT):
                    w1_m = fwp.tile([128, DT, 128], BF16, tag="w1_m")
                    nc.sync.dma_start(
                        w1_m[:],
                        w1[:, m * 128:(m + 1) * 128].rearrange(
                            "(ko p) n -> p ko n", p=128))
                    ps = fps.tile([128, 512], F32, tag="f_ps")
                    for k in range(DT):
                        nc.tensor.matmul(
                            ps[:], lhsT=w1_m[:, k, :], rhs=x2T[:, k, :],
                            start=(k == 0), stop=(k == DT - 1))
                    nc.vector.tensor_scalar(
                        out=h_sb[:, m, :], in0=ps[:],
                        scalar1=b1_sb[:, m:m + 1], scalar2=0.0,
                        op0=OP.add, op1=OP.max)
                    nc.scalar.activation(
                        out=h_sb[:, m, :], in_=h_sb[:, m, :], func=AF.Square)
                for m in range(DT):
                    w2_m = fwp.tile([128, FT, 128], BF16, tag="w2_m")
                    nc.sync.dma_start(
                        w2_m[:],
                        w2[:, m * 128:(m + 1) * 128].rearrange(
                            "(ko p) n -> p ko n", p=128))
                    ps = fps.tile([128, 512], F32, tag="f_ps")
                    for k in range(FT):
                        nc.tensor.matmul(
                            ps[:], lhsT=w2_m[:, k, :], rhs=h_sb[:, k, :],
                            start=(k == 0), stop=(k == FT - 1))
                    y2t = fp.tile([128, 512], BF16, tag="y2t")
                    nc.vector.tensor_scalar_add(y2t[:], ps[:], b2_sb[:, m:m + 1])
                    nc.sync.dma_start(s_y2T[m * 128:(m + 1) * 128, :], y2t[:])
                for t in range(QT):
                    y2_b = fp.tile([128, D], BF16, tag="y2_b")
                    nc.sync.dma_start_transpose(
                        y2_b[:], s_y2T[:, t * 128:(t + 1) * 128])
                    out_t = fp.tile([128, D], F32, tag="out_t")
                    nc.vector.tensor_tensor(out_t[:], x1[:, t, :], y2_b[:], OP.add)
                    nc.sync.dma_start(yout[t * 128:(t + 1) * 128, :], out_t[:])

    nc.compile()
    return nc


def make_inputs(inputs, core):
    """Build the per-core input map from full inputs. core = 2*b + j."""
    bf = ml_dtypes.bfloat16
    b, j = core // 2, core % 2
    x = np.asarray(inputs["x"], np.float32)
    memory = np.asarray(inputs["memory"], np.float32)
    nmr = np.asarray(inputs["nmr"], np.float32)
    g1 = np.asarray(inputs["ln1_g"], np.float32)
    b1n = np.asarray(inputs["ln1_b"], np.float32)
    g2 = np.asarray(inputs["ln2_g"], np.float32)
    b2n = np.asarray(inputs["ln2_b"], np.float32)

    def fold1(w, bias):
        wf = np.asarray(w, np.float32)
        bb = np.asarray(bias, np.float32)
        return (wf * g1[:, None]).astype(bf), (bb + b1n @ wf).astype(np.float32)

    wq_, bq_ = fold1(inputs["Wq"], inputs["bq"])
    wk_, bk_ = fold1(inputs["Wk"], inputs["bk"])
    wv_, bv_ = fold1(inputs["Wv"], inputs["bv"])
    w1f = np.asarray(inputs["W1"], np.float32)
    w1_ = (w1f * g2[:, None]).astype(bf)
    b1_ = (np.asarray(inputs["b1"], np.float32) + b2n @ w1f).astype(np.float32)
    wo_ = np.asarray(inputs["Wo"], np.float32).astype(bf)
    bo_ = np.asarray(inputs["bo"], np.float32)
    w2_ = np.asarray(inputs["W2"], np.float32).astype(bf)
    b2_ = np.asarray(inputs["b2"], np.float32)

    xin = np.zeros((LP, D), np.float32)
    xin[:M] = memory[b]
    xin[M:PREF] = nmr[b]
    xin[PAD_PREF:] = x[b]
    xq = np.ascontiguousarray(x[b, j * TQ:(j + 1) * TQ])

    # rope tables (feature-major rows; rows r%64 in [0,32) are rope dims)
    r = np.arange(128)
    d_loc = r % 64
    is_rope = d_loc < ROT
    inv_freq = 1.0 / (10000.0 ** (np.arange(0, ROT, 2, dtype=np.float32) / ROT))
    freq_row = np.where(is_rope, inv_freq[(d_loc % 16)], 0.0)   # [128]

    pos_k = np.arange(LP, dtype=np.float32)
    pos_k[PREF:PAD_PREF] = 0.0
    pos_k[PAD_PREF:] = PREF + np.arange(T)
    pos_q = PREF + j * TQ + np.arange(TQ, dtype=np.float32)

    sgn = np.where((d_loc % 32) < 16, -1.0, 1.0)  # rotate-half sign on sin

    def tables(pos):
        ang = freq_row[:, None] * pos[None, :]
        cos = np.where(is_rope[:, None], np.cos(ang), 1.0).astype(bf)
        sin = np.where(is_rope[:, None], sgn[:, None] * np.sin(ang), 0.0).astype(bf)
        return np.ascontiguousarray(cos), np.ascontiguousarray(sin)

    cosk_, sink_ = tables(pos_k)
    cosq_, sinq_ = tables(pos_q)

    # mask [128, KK, TQ]
    key = (np.arange(128)[:, None] + 128 * np.arange(KK)[None, :])  # [128, KK]
    qg = j * TQ + np.arange(TQ)                                     # [TQ]
    mask = np.zeros((128, KK, TQ), np.float32)
    prefix_ok = np.broadcast_to((key < PREF)[:, :, None], mask.shape)
    tk = key - PAD_PREF
    causal_ok = (key >= PAD_PREF)[:, :, None] & (tk[:, :, None] <= qg[None, None, :])
    mask[prefix_ok | causal_ok] = 1.0

    def bias_p(bias, nt):
        return np.ascontiguousarray(bias.reshape(nt, 128).T).astype(np.float32)

    return {
        "xin": xin, "xq": xq,
        "wq": wq_, "wk": wk_, "wv": wv_, "wo": wo_, "w1": w1_, "w2": w2_,
        "bq": bias_p(bq_, DT), "bk": bias_p(bk_, DT),
        "bv_r": bv_.reshape(1, D).astype(bf),
        "bo": bias_p(bo_, DT), "b1": bias_p(b1_, FT), "b2": bias_p(b2_, DT),
        "cosq": cosq_, "sinq": sinq_, "cosk": cosk_, "sink": sink_,
        "maskt": mask.astype(bf),
    }


_NC_CACHE = {}


def get_nc():
    if "nc" not in _NC_CACHE:
        _NC_CACHE["nc"] = build_kernel()
    return _NC_CACHE["nc"]


def kernel(**inputs) -> np.ndarray:
    from concourse.bass_utils import run_bass_kernel_spmd
    nc = get_nc()
    in_maps = [make_inputs(inputs, c) for c in range(8)]
    res = run_bass_kernel_spmd(nc, in_maps, list(range(8)))
    out = np.zeros((B, T, D), np.float32)
    for c in range(8):
        b, j = c // 2, c % 2
        out[b, j * TQ:(j + 1) * TQ] = res.results[c]["y"]
    return out


if __name__ == "__main__":
    nc = build_kernel()
    print("built ok")
